# revision 9
# baseline (speedup 1.0000x reference)
"""Causal single-head attention on 8 trn2 NeuronCores, data-parallel over batch.

Per core (one batch element, C=2048 ctx, E=1024 emb, D=1024 query_dim):
  P_X: transpose x -> xT (PE transpose, fp32), round to f32r, keep resident.
  P_Q: qT = (Qw^T @ x^T) + Qb   -> DRAM scratch (f32r), streamed back per row-block.
  P_K: kT = (Kw^T @ x^T) + Kb   -> DRAM scratch (f32r), streamed back in phase A.
  P_V: v  = (x @ Vw) + Vb        -> resident SBUF (f32r).
  A:   per 128-row block i: scores = qT_i^T @ kT (causal range only), mask diag,
       E = exp(scale*scores) with fused row-sum, PE-transpose E, out = E^T^T... @ v
       accumulated in PSUM, scaled by 1/rowsum, stored.

All matmuls use float32r (full PE rate at moving dim >= 256, ~2e-4 rel err).
"""

import os
import sys

for _p in ("/opt/trn_rl_repo", "/root/.axon_site/_ro/trn_rl_repo"):
    if os.path.isdir(_p) and _p not in sys.path:
        sys.path.insert(0, _p)

from contextlib import ExitStack

import numpy as np

import concourse.bass as bass
import concourse.tile as tile
from concourse import bacc, mybir
from concourse.bass_utils import run_bass_kernel_spmd
from concourse.masks import make_causal_mask, make_identity

F32 = mybir.dt.float32
F32R = mybir.dt.float32r
AF = mybir.ActivationFunctionType

P = 128


def build(C=2048, E=1024, D=1024, n_cores=8):
    CC = 512            # c-chunk width for projection passes (moving dim)
    NJ = 512            # c2 (key) chunk width in attention
    NCC = C // CC
    EC = E // P         # contraction chunks for projections
    DC = D // P
    RB = C // P         # number of 128-row query blocks
    ND = D // NJ        # out-matmul free-dim halves
    scale = float(D) ** -0.5

    nc = bacc.Bacc("TRN2", target_bir_lowering=False, debug=False,
                   num_devices=n_cores)
    x_d = nc.dram_tensor("x", [C, E], F32, kind="ExternalInput").ap()
    qw_d = nc.dram_tensor("Qw", [E, D], F32, kind="ExternalInput").ap()
    qb_d = nc.dram_tensor("Qb", [D], F32, kind="ExternalInput").ap()
    kw_d = nc.dram_tensor("Kw", [E, D], F32, kind="ExternalInput").ap()
    kb_d = nc.dram_tensor("Kb", [D], F32, kind="ExternalInput").ap()
    vw_d = nc.dram_tensor("Vw", [E, D], F32, kind="ExternalInput").ap()
    vb_d = nc.dram_tensor("Vb", [D], F32, kind="ExternalInput").ap()
    out_d = nc.dram_tensor("out", [C, D], F32, kind="ExternalOutput").ap()
    qt_d = nc.dram_tensor("qt_scratch", [RB, DC, P, P], F32R, kind="Internal").ap()
    kt_d = nc.dram_tensor("kt_scratch", [DC, P, C], F32R, kind="Internal").ap()

    with tile.TileContext(nc) as tc, ExitStack() as ctx:
        const_pool = ctx.enter_context(tc.tile_pool(name="const", bufs=1))
        v_pool = ctx.enter_context(tc.tile_pool(name="v", bufs=1))

        # ---- constants
        ident_f = const_pool.tile([P, P], F32, name="ident_f")
        make_identity(nc, ident_f)
        ident_r = const_pool.tile([P, P], F32R, name="ident_r")
        nc.vector.tensor_copy(ident_r[:], ident_f[:])
        cmask = const_pool.tile([P, P], F32, name="cmask")
        make_causal_mask(nc, cmask, mask_val=-1e9)
        ones_f = const_pool.tile([1, P], F32, name="ones_f")
        nc.vector.memset(ones_f[:], 1.0)
        ones_r = const_pool.tile([1, P], F32R, name="ones_r")
        nc.vector.tensor_copy(ones_r[:], ones_f[:])
        vb_f = const_pool.tile([1, D], F32, name="vb_f")
        nc.sync.dma_start(vb_f[:], vb_d[None, :])
        vb_r = const_pool.tile([1, D], F32R, name="vb_r")
        nc.vector.tensor_copy(vb_r[:], vb_f[:])
        qb_t = const_pool.tile([P, DC], F32, name="qb_t")
        nc.sync.dma_start(qb_t[:], qb_d.rearrange("(c p) -> p c", p=P))
        kb_t = const_pool.tile([P, DC], F32, name="kb_t")
        nc.sync.dma_start(kb_t[:], kb_d.rearrange("(c p) -> p c", p=P))

        # ---- resident tensors
        v_sb = [v_pool.tile([P, D], F32R, name=f"v{i}") for i in range(RB)]

        with tc.tile_pool(name="xt", bufs=1) as xt_pool:
            xt = [xt_pool.tile([P, C], F32R, name=f"xt{e}") for e in range(EC)]

            # ---- P_X: load + transpose x into xt (f32r)
            with tc.tile_pool(name="px_in", bufs=6) as xin_pool, \
                 tc.tile_pool(name="px_ps", bufs=4, space="PSUM") as pxps_pool:
                for cc in range(NCC):
                    xrows = []
                    for cs in range(CC // P):
                        xrow = xin_pool.tile([P, E], F32, tag="xrow")
                        nc.sync.dma_start(
                            xrow[:],
                            x_d[cc * CC + cs * P: cc * CC + (cs + 1) * P, :])
                        xrows.append(xrow)
                    for e in range(EC):
                        pst = pxps_pool.tile([P, CC], F32, tag="pst")
                        for cs in range(CC // P):
                            nc.tensor.transpose(
                                pst[:, cs * P:(cs + 1) * P],
                                xrows[cs][:, e * P:(e + 1) * P], ident_f[:])
                        nc.scalar.copy(xt[e][:, cc * CC:(cc + 1) * CC], pst[:])

            with tc.tile_pool(name="w", bufs=1) as w_pool:

                def load_w(w_dram, pname, st_pool):
                    w_sb = []
                    for e in range(EC):
                        wst = st_pool.tile([P, D], F32, tag="wst",
                                           name=f"{pname}st{e}")
                        nc.sync.dma_start(wst[:], w_dram[e * P:(e + 1) * P, :])
                        wt = w_pool.tile([P, D], F32R, tag=f"w{e}",
                                         name=f"{pname}{e}")
                        nc.vector.tensor_copy(wt[:], wst[:])
                        w_sb.append(wt)
                    return w_sb

                # ---- P_Q: qT tiles -> DRAM scratch
                with tc.tile_pool(name="pq_ps", bufs=6, space="PSUM") as ps_pool, \
                     tc.tile_pool(name="pq_st", bufs=3) as st_pool, \
                     tc.tile_pool(name="pq_w", bufs=2) as wstp:
                    qw_sb = load_w(qw_d, "qw", wstp)
                    for cc in range(NCC):
                        for dc in range(DC):
                            ps = ps_pool.tile([P, CC], F32, tag="ps")
                            for e in range(EC):
                                nc.tensor.matmul(
                                    ps[:], qw_sb[e][:, dc * P:(dc + 1) * P],
                                    xt[e][:, cc * CC:(cc + 1) * CC],
                                    start=(e == 0), stop=(e == EC - 1))
                            qst = st_pool.tile([P, CC], F32R, tag="qst")
                            nc.scalar.activation(qst[:], ps[:], AF.Identity,
                                                 bias=qb_t[:, dc:dc + 1])
                            for rb in range(CC // P):
                                nc.sync.dma_start(
                                    qt_d[cc * (CC // P) + rb, dc],
                                    qst[:, rb * P:(rb + 1) * P])

                # ---- P_K: kT tiles -> DRAM scratch
                with tc.tile_pool(name="pk_ps", bufs=6, space="PSUM") as ps_pool, \
                     tc.tile_pool(name="pk_st", bufs=3) as st_pool, \
                     tc.tile_pool(name="pk_w", bufs=2) as wstp:
                    kw_sb = load_w(kw_d, "kw", wstp)
                    for cc in range(NCC):
                        for dc in range(DC):
                            ps = ps_pool.tile([P, CC], F32, tag="ps")
                            for e in range(EC):
                                nc.tensor.matmul(
                                    ps[:], kw_sb[e][:, dc * P:(dc + 1) * P],
                                    xt[e][:, cc * CC:(cc + 1) * CC],
                                    start=(e == 0), stop=(e == EC - 1))
                            kst = st_pool.tile([P, CC], F32R, tag="kst")
                            nc.scalar.activation(kst[:], ps[:], AF.Identity,
                                                 bias=kb_t[:, dc:dc + 1])
                            nc.sync.dma_start(
                                kt_d[dc, :, cc * CC:(cc + 1) * CC], kst[:])

                # ---- P_V: v resident (natural layout)
                with tc.tile_pool(name="pv_ps", bufs=6, space="PSUM") as ps_pool, \
                     tc.tile_pool(name="pv_w", bufs=2) as wstp:
                    vw_sb = load_w(vw_d, "vw", wstp)
                    for ct in range(RB):
                        for dh in range(ND):
                            ps = ps_pool.tile([P, NJ], F32, tag="ps")
                            for e in range(EC):
                                nc.tensor.matmul(
                                    ps[:], xt[e][:, ct * P:(ct + 1) * P],
                                    vw_sb[e][:, dh * NJ:(dh + 1) * NJ],
                                    start=(e == 0), stop=False)
                            nc.tensor.matmul(ps[:], ones_r[:],
                                             vb_r[:, dh * NJ:(dh + 1) * NJ],
                                             start=False, stop=True)
                            nc.scalar.copy(v_sb[ct][:, dh * NJ:(dh + 1) * NJ],
                                           ps[:])

        # ---- Phase A: causal attention per row-block
        with tc.tile_pool(name="kt", bufs=1) as kt_pool, \
             tc.tile_pool(name="q", bufs=2) as q_pool, \
             tc.tile_pool(name="e", bufs=2) as e_pool, \
             tc.tile_pool(name="et", bufs=2) as et_pool, \
             tc.tile_pool(name="r", bufs=2) as r_pool, \
             tc.tile_pool(name="os", bufs=2) as os_pool, \
             tc.tile_pool(name="a_s", bufs=2, space="PSUM") as s_pool, \
             tc.tile_pool(name="a_t", bufs=2, space="PSUM") as t_pool, \
             tc.tile_pool(name="a_o", bufs=2, space="PSUM") as o_pool:
            NKC = C // NJ
            kt = [[kt_pool.tile([P, NJ], F32R, name=f"kt{d}_{j}")
                   for j in range(NKC)] for d in range(DC)]
            for j in range(NKC):
                for d in range(DC):
                    nc.sync.dma_start(kt[d][j][:],
                                      kt_d[d, :, j * NJ:(j + 1) * NJ])

            for i in range(RB):
                ncols = (i + 1) * P
                njj = (ncols + NJ - 1) // NJ
                qx = q_pool.tile([P, DC * P], F32R, tag="qx")
                for d in range(DC):
                    nc.sync.dma_start(qx[:, d * P:(d + 1) * P], qt_d[i, d])

                etile = e_pool.tile([P, C], F32R, tag="E")
                acc = r_pool.tile([P, NKC], F32, tag="acc")
                for jj in range(njj):
                    n = min(NJ, ncols - jj * NJ)
                    ps_s = s_pool.tile([P, NJ], F32, tag="ps_s")
                    for d in range(DC):
                        nc.tensor.matmul(
                            ps_s[:, :n], qx[:, d * P:(d + 1) * P],
                            kt[d][jj][:, :n],
                            start=(d == 0), stop=(d == DC - 1))
                    if jj == njj - 1:
                        dcol = i * P - jj * NJ
                        nc.vector.tensor_add(ps_s[:, dcol:dcol + P],
                                             ps_s[:, dcol:dcol + P], cmask[:])
                    nc.scalar.activation(
                        etile[:, jj * NJ:jj * NJ + n], ps_s[:, :n], AF.Exp,
                        scale=scale, accum_out=acc[:, jj:jj + 1])

                rs = r_pool.tile([P, 1], F32, tag="rs")
                nc.vector.reduce_sum(rs[:], acc[:, :njj],
                                     axis=mybir.AxisListType.X)
                rinv = r_pool.tile([P, 1], F32, tag="rinv")
                nc.vector.reciprocal(rinv[:], rs[:])

                ettile = et_pool.tile([P, C], F32R, tag="ET")
                for jj in range(njj):
                    n = min(NJ, ncols - jj * NJ)
                    ps_t = t_pool.tile([P, NJ], F32R, tag="ps_t")
                    for j in range(n // P):
                        nc.tensor.transpose(
                            ps_t[:, j * P:(j + 1) * P],
                            etile[:, jj * NJ + j * P: jj * NJ + (j + 1) * P],
                            ident_r[:])
                    nc.vector.tensor_copy(ettile[:, jj * NJ:jj * NJ + n],
                                          ps_t[:, :n])

                ps_o = o_pool.tile([P, D], F32, tag="ps_o")
                for j in range(i + 1):
                    for dh in range(ND):
                        nc.tensor.matmul(
                            ps_o[:, dh * NJ:(dh + 1) * NJ],
                            ettile[:, j * P:(j + 1) * P],
                            v_sb[j][:, dh * NJ:(dh + 1) * NJ],
                            start=(j == 0), stop=(j == i))
                outst = os_pool.tile([P, D], F32, tag="outst")
                nc.vector.tensor_scalar_mul(outst[:], ps_o[:], rinv[:])
                nc.sync.dma_start(out_d[i * P:(i + 1) * P, :], outst[:])

    nc.compile()
    return nc


_CACHE = {}


def _built(C=2048, E=1024, D=1024, n_cores=8):
    key = (C, E, D, n_cores)
    if key not in _CACHE:
        _CACHE[key] = build(C, E, D, n_cores)
    return _CACHE[key]


def run(inputs, C=2048, E=1024, D=1024, n_cores=8, trace=False):
    nc = _built(C, E, D, n_cores)
    B = inputs["x"].shape[0]
    assert B == n_cores
    f = lambda a: np.ascontiguousarray(np.asarray(a, dtype=np.float32))
    shared = {k: f(inputs[k]) for k in ("Qw", "Qb", "Kw", "Kb", "Vw", "Vb")}
    x = f(inputs["x"])
    in_maps = [dict(x=x[b], **shared) for b in range(B)]
    res = run_bass_kernel_spmd(nc, in_maps, list(range(n_cores)), trace=trace)
    out = np.stack([res.results[b]["out"] for b in range(B)], axis=0)
    return out, res


def kernel(**inputs) -> np.ndarray:
    out, _ = run(inputs)
    return out


# revision 11
# speedup vs baseline: 55.5890x; 55.5890x over previous
"""Causal single-head attention on 8 trn2 NeuronCores, data-parallel over batch.

Per core (one batch element, C=2048 ctx, E=1024 emb, D=1024 query_dim):
  P_X: transpose x -> xT (PE transpose, fp32), round to f32r, keep resident.
  P_Q: qT = (Qw^T @ x^T) + Qb   -> DRAM scratch (f32r), streamed back per row-block.
  P_K: kT = (Kw^T @ x^T) + Kb   -> DRAM scratch (f32r), streamed back in phase A.
  P_V: v  = (x @ Vw) + Vb        -> resident SBUF (f32r).
  A:   per 128-row block i: scores = qT_i^T @ kT (causal range only), mask diag,
       E = exp(scale*scores) with fused row-sum, PE-transpose E, out = E^T^T... @ v
       accumulated in PSUM, scaled by 1/rowsum, stored.

All matmuls use float32r (full PE rate at moving dim >= 256, ~2e-4 rel err).
"""

import os
import sys

for _p in ("/opt/trn_rl_repo", "/root/.axon_site/_ro/trn_rl_repo"):
    if os.path.isdir(_p) and _p not in sys.path:
        sys.path.insert(0, _p)

from contextlib import ExitStack

import numpy as np

import concourse.bass as bass
import concourse.tile as tile
from concourse import bacc, mybir
from concourse.bass_utils import run_bass_kernel_spmd
from concourse.masks import make_causal_mask, make_identity

F32 = mybir.dt.float32
F32R = mybir.dt.float32r
AF = mybir.ActivationFunctionType

P = 128


def build(C=2048, E=1024, D=1024, n_cores=8):
    CC = 512            # c-chunk width for projection passes (moving dim)
    NJ = 512            # c2 (key) chunk width in attention
    NCC = C // CC
    EC = E // P         # contraction chunks for projections
    DC = D // P
    RB = C // P         # number of 128-row query blocks
    ND = D // NJ        # out-matmul free-dim halves
    scale = float(D) ** -0.5

    nc = bacc.Bacc("TRN2", target_bir_lowering=False, debug=False,
                   num_devices=n_cores)
    x_d = nc.dram_tensor("x", [C, E], F32, kind="ExternalInput").ap()
    qw_d = nc.dram_tensor("Qw", [E, D], F32, kind="ExternalInput").ap()
    qb_d = nc.dram_tensor("Qb", [D], F32, kind="ExternalInput").ap()
    kw_d = nc.dram_tensor("Kw", [E, D], F32, kind="ExternalInput").ap()
    kb_d = nc.dram_tensor("Kb", [D], F32, kind="ExternalInput").ap()
    vw_d = nc.dram_tensor("Vw", [E, D], F32, kind="ExternalInput").ap()
    vb_d = nc.dram_tensor("Vb", [D], F32, kind="ExternalInput").ap()
    out_d = nc.dram_tensor("out", [C, D], F32, kind="ExternalOutput").ap()
    qt_d = nc.dram_tensor("qt_scratch", [RB, DC, P, P], F32R, kind="Internal").ap()
    kt_d = nc.dram_tensor("kt_scratch", [DC, P, C], F32R, kind="Internal").ap()

    with tile.TileContext(nc) as tc, ExitStack() as ctx:
        const_pool = ctx.enter_context(tc.tile_pool(name="const", bufs=1))
        v_pool = ctx.enter_context(tc.tile_pool(name="v", bufs=1))

        # ---- constants
        ident_f = const_pool.tile([P, P], F32, name="ident_f")
        make_identity(nc, ident_f)
        ident_r = const_pool.tile([P, P], F32R, name="ident_r")
        nc.vector.tensor_copy(ident_r[:], ident_f[:])
        cmask = const_pool.tile([P, P], F32, name="cmask")
        make_causal_mask(nc, cmask, mask_val=-1e9)
        ones_f = const_pool.tile([1, P], F32, name="ones_f")
        nc.vector.memset(ones_f[:], 1.0)
        ones_r = const_pool.tile([1, P], F32R, name="ones_r")
        nc.vector.tensor_copy(ones_r[:], ones_f[:])
        vb_f = const_pool.tile([1, D], F32, name="vb_f")
        nc.sync.dma_start(vb_f[:], vb_d[None, :])
        vb_r = const_pool.tile([1, D], F32R, name="vb_r")
        nc.vector.tensor_copy(vb_r[:], vb_f[:])
        qb_t = const_pool.tile([P, DC], F32, name="qb_t")
        nc.sync.dma_start(qb_t[:], qb_d.rearrange("(c p) -> p c", p=P))
        kb_t = const_pool.tile([P, DC], F32, name="kb_t")
        nc.sync.dma_start(kb_t[:], kb_d.rearrange("(c p) -> p c", p=P))

        # ---- resident tensors
        v_sb = [v_pool.tile([P, D], F32R, name=f"v{i}") for i in range(RB)]

        with tc.tile_pool(name="xt", bufs=1) as xt_pool:
            xt = [xt_pool.tile([P, C], F32R, name=f"xt{e}") for e in range(EC)]

            # ---- P_X: load + transpose x into xt (f32r)
            with tc.tile_pool(name="px_in", bufs=6) as xin_pool, \
                 tc.tile_pool(name="px_ps", bufs=4, space="PSUM") as pxps_pool:
                for cc in range(NCC):
                    xrows = []
                    for cs in range(CC // P):
                        xrow = xin_pool.tile([P, E], F32, tag="xrow")
                        nc.sync.dma_start(
                            xrow[:],
                            x_d[cc * CC + cs * P: cc * CC + (cs + 1) * P, :])
                        xrows.append(xrow)
                    for e in range(EC):
                        pst = pxps_pool.tile([P, CC], F32, tag="pst")
                        for cs in range(CC // P):
                            nc.tensor.transpose(
                                pst[:, cs * P:(cs + 1) * P],
                                xrows[cs][:, e * P:(e + 1) * P], ident_f[:])
                        nc.scalar.copy(xt[e][:, cc * CC:(cc + 1) * CC], pst[:])

            with tc.tile_pool(name="w", bufs=1) as w_pool:

                def load_w(w_dram, pname, st_pool):
                    w_sb = []
                    for e in range(EC):
                        wst = st_pool.tile([P, D], F32, tag="wst",
                                           name=f"{pname}st{e}")
                        nc.sync.dma_start(wst[:], w_dram[e * P:(e + 1) * P, :])
                        wt = w_pool.tile([P, D], F32R, tag=f"w{e}",
                                         name=f"{pname}{e}")
                        nc.vector.tensor_copy(wt[:], wst[:])
                        w_sb.append(wt)
                    return w_sb

                # ---- P_Q: qT tiles -> DRAM scratch
                with tc.tile_pool(name="pq_ps", bufs=6, space="PSUM") as ps_pool, \
                     tc.tile_pool(name="pq_st", bufs=3) as st_pool, \
                     tc.tile_pool(name="pq_w", bufs=2) as wstp:
                    qw_sb = load_w(qw_d, "qw", wstp)
                    for cc in range(NCC):
                        for dc in range(DC):
                            ps = ps_pool.tile([P, CC], F32, tag="ps")
                            for e in range(EC):
                                nc.tensor.matmul(
                                    ps[:], qw_sb[e][:, dc * P:(dc + 1) * P],
                                    xt[e][:, cc * CC:(cc + 1) * CC],
                                    start=(e == 0), stop=(e == EC - 1))
                            qst = st_pool.tile([P, CC], F32R, tag="qst")
                            nc.scalar.activation(qst[:], ps[:], AF.Identity,
                                                 bias=qb_t[:, dc:dc + 1])
                            for rb in range(CC // P):
                                nc.sync.dma_start(
                                    qt_d[cc * (CC // P) + rb, dc],
                                    qst[:, rb * P:(rb + 1) * P])

                # ---- P_K: kT tiles -> DRAM scratch
                with tc.tile_pool(name="pk_ps", bufs=6, space="PSUM") as ps_pool, \
                     tc.tile_pool(name="pk_st", bufs=3) as st_pool, \
                     tc.tile_pool(name="pk_w", bufs=2) as wstp:
                    kw_sb = load_w(kw_d, "kw", wstp)
                    for cc in range(NCC):
                        for dc in range(DC):
                            ps = ps_pool.tile([P, CC], F32, tag="ps")
                            for e in range(EC):
                                nc.tensor.matmul(
                                    ps[:], kw_sb[e][:, dc * P:(dc + 1) * P],
                                    xt[e][:, cc * CC:(cc + 1) * CC],
                                    start=(e == 0), stop=(e == EC - 1))
                            kst = st_pool.tile([P, CC], F32R, tag="kst")
                            nc.scalar.activation(kst[:], ps[:], AF.Identity,
                                                 bias=kb_t[:, dc:dc + 1])
                            nc.sync.dma_start(
                                kt_d[dc, :, cc * CC:(cc + 1) * CC], kst[:])

                # ---- P_V: v resident (natural layout)
                with tc.tile_pool(name="pv_ps", bufs=6, space="PSUM") as ps_pool, \
                     tc.tile_pool(name="pv_w", bufs=2) as wstp:
                    vw_sb = load_w(vw_d, "vw", wstp)
                    for ct in range(RB):
                        for dh in range(ND):
                            ps = ps_pool.tile([P, NJ], F32, tag="ps")
                            for e in range(EC):
                                nc.tensor.matmul(
                                    ps[:], xt[e][:, ct * P:(ct + 1) * P],
                                    vw_sb[e][:, dh * NJ:(dh + 1) * NJ],
                                    start=(e == 0), stop=False)
                            nc.tensor.matmul(ps[:], ones_r[:],
                                             vb_r[:, dh * NJ:(dh + 1) * NJ],
                                             start=False, stop=True)
                            nc.scalar.copy(v_sb[ct][:, dh * NJ:(dh + 1) * NJ],
                                           ps[:])

        # ---- Phase A: causal attention per row-block
        with tc.tile_pool(name="kt", bufs=1) as kt_pool, \
             tc.tile_pool(name="q", bufs=2) as q_pool, \
             tc.tile_pool(name="e", bufs=2) as e_pool, \
             tc.tile_pool(name="et", bufs=2) as et_pool, \
             tc.tile_pool(name="r", bufs=2) as r_pool, \
             tc.tile_pool(name="os", bufs=2) as os_pool, \
             tc.tile_pool(name="a_s", bufs=2, space="PSUM") as s_pool, \
             tc.tile_pool(name="a_t", bufs=2, space="PSUM") as t_pool, \
             tc.tile_pool(name="a_o", bufs=2, space="PSUM") as o_pool:
            NKC = C // NJ
            kt = [[kt_pool.tile([P, NJ], F32R, name=f"kt{d}_{j}")
                   for j in range(NKC)] for d in range(DC)]
            for j in range(NKC):
                for d in range(DC):
                    nc.sync.dma_start(kt[d][j][:],
                                      kt_d[d, :, j * NJ:(j + 1) * NJ])

            for i in range(RB):
                ncols = (i + 1) * P
                njj = (ncols + NJ - 1) // NJ
                qx = q_pool.tile([P, DC * P], F32R, tag="qx")
                for d in range(DC):
                    nc.sync.dma_start(qx[:, d * P:(d + 1) * P], qt_d[i, d])

                etile = e_pool.tile([P, C], F32R, tag="E")
                acc = r_pool.tile([P, NKC], F32, tag="acc")
                for jj in range(njj):
                    n = min(NJ, ncols - jj * NJ)
                    ps_s = s_pool.tile([P, NJ], F32, tag="ps_s")
                    for d in range(DC):
                        nc.tensor.matmul(
                            ps_s[:, :n], qx[:, d * P:(d + 1) * P],
                            kt[d][jj][:, :n],
                            start=(d == 0), stop=(d == DC - 1))
                    if jj == njj - 1:
                        dcol = i * P - jj * NJ
                        nc.vector.tensor_add(ps_s[:, dcol:dcol + P],
                                             ps_s[:, dcol:dcol + P], cmask[:])
                    nc.scalar.activation(
                        etile[:, jj * NJ:jj * NJ + n], ps_s[:, :n], AF.Exp,
                        scale=scale, accum_out=acc[:, jj:jj + 1])

                rs = r_pool.tile([P, 1], F32, tag="rs")
                nc.vector.reduce_sum(rs[:], acc[:, :njj],
                                     axis=mybir.AxisListType.X)
                rinv = r_pool.tile([P, 1], F32, tag="rinv")
                nc.vector.reciprocal(rinv[:], rs[:])

                ettile = et_pool.tile([P, C], F32R, tag="ET")
                for jj in range(njj):
                    n = min(NJ, ncols - jj * NJ)
                    ps_t = t_pool.tile([P, NJ], F32R, tag="ps_t")
                    for j in range(n // P):
                        nc.tensor.transpose(
                            ps_t[:, j * P:(j + 1) * P],
                            etile[:, jj * NJ + j * P: jj * NJ + (j + 1) * P],
                            ident_r[:])
                    nc.vector.tensor_copy(ettile[:, jj * NJ:jj * NJ + n],
                                          ps_t[:, :n])

                ps_o = o_pool.tile([P, D], F32, tag="ps_o")
                for j in range(i + 1):
                    for dh in range(ND):
                        nc.tensor.matmul(
                            ps_o[:, dh * NJ:(dh + 1) * NJ],
                            ettile[:, j * P:(j + 1) * P],
                            v_sb[j][:, dh * NJ:(dh + 1) * NJ],
                            start=(j == 0), stop=(j == i))
                outst = os_pool.tile([P, D], F32, tag="outst")
                nc.vector.tensor_scalar_mul(outst[:], ps_o[:], rinv[:])
                nc.sync.dma_start(out_d[i * P:(i + 1) * P, :], outst[:])

    nc.compile()
    return nc


_CACHE = {}


def _built(C=2048, E=1024, D=1024, n_cores=8):
    key = (C, E, D, n_cores)
    if key not in _CACHE:
        _CACHE[key] = build(C, E, D, n_cores)
    return _CACHE[key]


def _executable(C=2048, E=1024, D=1024, n_cores=8):
    """Cached jitted SPMD executable for the built Bass module.

    Replicates concourse.bass2jax.run_bass_via_pjrt's multi-core path but
    caches the jit so repeat calls don't retrace, and exposes the pieces
    needed for device-resident benchmarking.
    """
    key = ("exec", C, E, D, n_cores)
    if key in _CACHE:
        return _CACHE[key]
    import jax
    from jax.sharding import Mesh, PartitionSpec
    from jax.experimental.shard_map import shard_map
    from concourse import bass2jax, mybir as _mybir

    nc = _built(C, E, D, n_cores)
    bass2jax.install_neuronx_cc_hook()

    partition_name = (nc.partition_id_tensor.name
                      if nc.partition_id_tensor else None)
    in_names, out_names, out_avals, zero_outs = [], [], [], []
    for alloc in nc.m.functions[0].allocations:
        if not isinstance(alloc, _mybir.MemoryLocationSet):
            continue
        name = alloc.memorylocations[0].name
        if alloc.kind == "ExternalInput":
            if name != partition_name:
                in_names.append(name)
        elif alloc.kind == "ExternalOutput":
            out_names.append(name)
            shape = tuple(alloc.tensor_shape)
            dtype = _mybir.dt.np(alloc.dtype)
            out_avals.append(jax.core.ShapedArray(shape, dtype))
            zero_outs.append(np.zeros(shape, dtype))
    n_params = len(in_names)
    all_names = in_names + out_names
    if partition_name is not None:
        all_names = all_names + [partition_name]

    def _body(*args):
        operands = list(args)
        if partition_name is not None:
            operands.append(bass2jax.partition_id_tensor())
        outs = bass2jax._bass_exec_p.bind(
            *operands,
            out_avals=tuple(out_avals),
            in_names=tuple(all_names),
            out_names=tuple(out_names),
            lowering_input_output_aliases=(),
            sim_require_finite=True,
            sim_require_nnan=True,
            nc=nc,
        )
        return tuple(outs)

    devices = jax.devices()[:n_cores]
    mesh = Mesh(np.asarray(devices), ("core",))
    n_outs = len(out_names)
    sharded = jax.jit(
        shard_map(_body, mesh=mesh,
                  in_specs=(PartitionSpec("core"),) * (n_params + n_outs),
                  out_specs=(PartitionSpec("core"),) * n_outs,
                  check_rep=False),
        donate_argnums=tuple(range(n_params, n_params + n_outs)),
        keep_unused=True,
    )
    res = dict(fn=sharded, in_names=in_names, out_names=out_names,
               out_avals=out_avals, zero_outs=zero_outs, mesh=mesh,
               n_cores=n_cores)
    _CACHE[key] = res
    return res


def run(inputs, C=2048, E=1024, D=1024, n_cores=8):
    ex = _executable(C, E, D, n_cores)
    B = inputs["x"].shape[0]
    assert B == n_cores
    f = lambda a: np.ascontiguousarray(np.asarray(a, dtype=np.float32))
    shared = {k: f(inputs[k]) for k in ("Qw", "Qb", "Kw", "Kb", "Vw", "Vb")}
    x = f(inputs["x"])
    per_core = [dict(x=x[b], **shared) for b in range(B)]
    concat_in = [
        np.concatenate([per_core[c][n] for c in range(n_cores)], axis=0)
        for n in ex["in_names"]
    ]
    concat_zeros = [
        np.zeros((n_cores * z.shape[0], *z.shape[1:]), z.dtype)
        for z in ex["zero_outs"]
    ]
    out_arrs = ex["fn"](*concat_in, *concat_zeros)
    i = ex["out_names"].index("out")
    out = np.asarray(out_arrs[i]).reshape(n_cores, *ex["out_avals"][i].shape)
    return out


def kernel(**inputs) -> np.ndarray:
    return run(inputs)


# revision 23
# speedup vs baseline: 91.3076x; 1.6425x over previous
"""Causal single-head attention on 8 trn2 NeuronCores, data-parallel over batch.

Per core (one batch element, C=2048 ctx, E=1024 emb, D=1024 query_dim):
  P_X: transpose x -> xT (PE transpose, fp32), cast to DT, keep resident.
  P_Q: qT = (Qw^T @ x^T) + Qb   -> DRAM scratch (DT), streamed back per row-block.
  P_K: kT = (Kw^T @ x^T) + Kb   -> DRAM scratch (DT), streamed back in phase A.
  P_V: v  = (x @ Vw) + Vb        -> resident SBUF (DT).
  A:   per 128-row query block i: scores = qT_i^T @ kT (causal range only),
       additive -1e9 mask on the diagonal tile, E = exp(scale*scores) with
       fused row-sum on the scalar engine, PE-transpose E, out = sum_j E^T_j @ v_j
       accumulated in PSUM, scaled by 1/rowsum, stored.

DT is the matmul dtype: float16 (default), bfloat16, or float32r.
"""

import os
import sys

for _p in ("/opt/trn_rl_repo", "/root/.axon_site/_ro/trn_rl_repo"):
    if os.path.isdir(_p) and _p not in sys.path:
        sys.path.insert(0, _p)

from contextlib import ExitStack

import numpy as np

import concourse.bass as bass
import concourse.tile as tile
from concourse import bacc, mybir
from concourse.bass_utils import run_bass_kernel_spmd
from concourse.masks import make_causal_mask, make_identity

F32 = mybir.dt.float32
AF = mybir.ActivationFunctionType
DTYPES = {"fp16": mybir.dt.float16, "bf16": mybir.dt.bfloat16,
          "f32r": mybir.dt.float32r}

P = 128


def build(C=2048, E=1024, D=1024, n_cores=8, loop=1, dt="fp16"):
    DT = DTYPES[dt]
    CC = 512            # c-chunk width for projection passes
    NJ = 512            # matmul moving-dim width
    NCC = C // CC
    EC = E // P         # contraction chunks for projections
    DC = D // P
    RB = C // P         # number of 128-row blocks
    ND = D // NJ
    scale = float(D) ** -0.5

    nc = bacc.Bacc("TRN2", target_bir_lowering=False, debug=False,
                   num_devices=n_cores)
    x_d = nc.dram_tensor("x", [C, E], F32, kind="ExternalInput").ap()
    qw_d = nc.dram_tensor("Qw", [E, D], F32, kind="ExternalInput").ap()
    qb_d = nc.dram_tensor("Qb", [D], F32, kind="ExternalInput").ap()
    kw_d = nc.dram_tensor("Kw", [E, D], F32, kind="ExternalInput").ap()
    kb_d = nc.dram_tensor("Kb", [D], F32, kind="ExternalInput").ap()
    vw_d = nc.dram_tensor("Vw", [E, D], F32, kind="ExternalInput").ap()
    vb_d = nc.dram_tensor("Vb", [D], F32, kind="ExternalInput").ap()
    out_d = nc.dram_tensor("out", [C, D], F32, kind="ExternalOutput").ap()
    qt_d = nc.dram_tensor("qt_scratch", [RB, DC, P, P], DT, kind="Internal").ap()
    kt_d = nc.dram_tensor("kt_scratch", [DC, P, C], DT, kind="Internal").ap()

    with tile.TileContext(nc) as tc, ExitStack() as ctx:
        if loop > 1:
            ctx.enter_context(tc.For_i(0, loop, 1))
        const_pool = ctx.enter_context(tc.tile_pool(name="const", bufs=1))
        v_pool = ctx.enter_context(tc.tile_pool(name="v", bufs=1))

        # ---- constants
        ident_f = const_pool.tile([P, P], F32, name="ident_f")
        make_identity(nc, ident_f)
        ident_h = const_pool.tile([P, P], DT, name="ident_h")
        nc.vector.tensor_copy(ident_h[:], ident_f[:])
        cmask = const_pool.tile([P, P], F32, name="cmask")
        make_causal_mask(nc, cmask, mask_val=-1e9)
        ones_f = const_pool.tile([1, P], F32, name="ones_f")
        nc.vector.memset(ones_f[:], 1.0)
        ones_h = const_pool.tile([1, P], DT, name="ones_h")
        nc.vector.tensor_copy(ones_h[:], ones_f[:])
        vb_f = const_pool.tile([1, D], F32, name="vb_f")
        nc.sync.dma_start(vb_f[:], vb_d[None, :])
        vb_h = const_pool.tile([1, D], DT, name="vb_h")
        nc.vector.tensor_copy(vb_h[:], vb_f[:])
        qb_t = const_pool.tile([P, DC], F32, name="qb_t")
        nc.sync.dma_start(qb_t[:], qb_d.rearrange("(c p) -> p c", p=P))
        kb_t = const_pool.tile([P, DC], F32, name="kb_t")
        nc.sync.dma_start(kb_t[:], kb_d.rearrange("(c p) -> p c", p=P))

        # ---- resident v
        v_sb = [v_pool.tile([P, D], DT, name=f"v{i}") for i in range(RB)]

        with tc.tile_pool(name="xt", bufs=1) as xt_pool:
            xt = [xt_pool.tile([P, C], DT, name=f"xt{e}") for e in range(EC)]

            # ---- P_X: load + transpose x into xt
            with tc.tile_pool(name="px_in", bufs=6) as xin_pool, \
                 tc.tile_pool(name="px_ps", bufs=4, space="PSUM") as pxps_pool:
                for cc in range(NCC):
                    xrows = []
                    for cs in range(CC // P):
                        xrow = xin_pool.tile([P, E], F32, tag="xrow")
                        nc.sync.dma_start(
                            xrow[:],
                            x_d[cc * CC + cs * P: cc * CC + (cs + 1) * P, :])
                        xrows.append(xrow)
                    for e in range(EC):
                        pst = pxps_pool.tile([P, CC], F32, tag="pst")
                        for cs in range(CC // P):
                            nc.tensor.transpose(
                                pst[:, cs * P:(cs + 1) * P],
                                xrows[cs][:, e * P:(e + 1) * P], ident_f[:])
                        nc.scalar.copy(xt[e][:, cc * CC:(cc + 1) * CC], pst[:])

            with tc.tile_pool(name="w", bufs=1) as w_pool:

                def load_w(w_dram, pname, st_pool):
                    w_sb = []
                    for e in range(EC):
                        wst = st_pool.tile([P, D], F32, tag="wst",
                                           name=f"{pname}st{e}")
                        nc.sync.dma_start(wst[:], w_dram[e * P:(e + 1) * P, :])
                        wt = w_pool.tile([P, D], DT, tag=f"w{e}",
                                         name=f"{pname}{e}")
                        nc.vector.tensor_copy(wt[:], wst[:])
                        w_sb.append(wt)
                    return w_sb

                # ---- P_Q / P_K: qT/kT tiles -> DRAM scratch.
                # Consecutive matmuls rotate the stationary operand
                # (same-stationary back-to-back measures 2x slower).
                def proj_t(w_sb, bias_t, store):
                    with tc.tile_pool(name="pp_ps", bufs=6, space="PSUM") \
                            as ps_pool, \
                         tc.tile_pool(name="pp_st", bufs=3) as st_pool:
                        for cc in range(NCC):
                            for dc in range(DC):
                                ps = ps_pool.tile([P, CC], F32, tag="ps")
                                for e in range(EC):
                                    nc.tensor.matmul(
                                        ps[:],
                                        w_sb[e][:, dc * P:(dc + 1) * P],
                                        xt[e][:, cc * CC:(cc + 1) * CC],
                                        start=(e == 0), stop=(e == EC - 1))
                                pst = st_pool.tile([P, CC], DT, tag="pst")
                                nc.scalar.activation(
                                    pst[:], ps[:], AF.Identity,
                                    bias=bias_t[:, dc:dc + 1])
                                store(dc, cc, pst)

                with tc.tile_pool(name="pq_w", bufs=2) as wstp:
                    qw_sb = load_w(qw_d, "qw", wstp)

                    def store_q(dc, cc, pst):
                        for rb in range(CC // P):
                            nc.sync.dma_start(
                                qt_d[cc * (CC // P) + rb, dc],
                                pst[:, rb * P:(rb + 1) * P])
                    proj_t(qw_sb, qb_t, store_q)

                with tc.tile_pool(name="pk_w", bufs=2) as wstp:
                    kw_sb = load_w(kw_d, "kw", wstp)

                    def store_k(dc, cc, pst):
                        nc.sync.dma_start(
                            kt_d[dc, :, cc * CC:(cc + 1) * CC], pst[:])
                    proj_t(kw_sb, kb_t, store_k)

                # ---- P_V: v resident (natural layout)
                with tc.tile_pool(name="pv_ps", bufs=6, space="PSUM") \
                        as ps_pool, \
                     tc.tile_pool(name="pv_w", bufs=2) as wstp:
                    vw_sb = load_w(vw_d, "vw", wstp)
                    for ct in range(RB):
                        for dh in range(ND):
                            ps = ps_pool.tile([P, NJ], F32, tag="ps")
                            for e in range(EC):
                                nc.tensor.matmul(
                                    ps[:], xt[e][:, ct * P:(ct + 1) * P],
                                    vw_sb[e][:, dh * NJ:(dh + 1) * NJ],
                                    start=(e == 0), stop=False)
                            nc.tensor.matmul(ps[:], ones_h[:],
                                             vb_h[:, dh * NJ:(dh + 1) * NJ],
                                             start=False, stop=True)
                            nc.scalar.copy(v_sb[ct][:, dh * NJ:(dh + 1) * NJ],
                                           ps[:])

        # ---- Phase A: causal attention per row-block
        with tc.tile_pool(name="kt", bufs=1) as kt_pool, \
             tc.tile_pool(name="q", bufs=2) as q_pool, \
             tc.tile_pool(name="e", bufs=2) as e_pool, \
             tc.tile_pool(name="et", bufs=2) as et_pool, \
             tc.tile_pool(name="r", bufs=2) as r_pool, \
             tc.tile_pool(name="os", bufs=2) as os_pool, \
             tc.tile_pool(name="a_s", bufs=2, space="PSUM") as s_pool, \
             tc.tile_pool(name="a_t", bufs=2, space="PSUM") as t_pool, \
             tc.tile_pool(name="a_o", bufs=2, space="PSUM") as o_pool:
            NKC = C // NJ
            kt = [[kt_pool.tile([P, NJ], DT, name=f"kt{d}_{j}")
                   for j in range(NKC)] for d in range(DC)]
            for j in range(NKC):
                for d in range(DC):
                    nc.sync.dma_start(kt[d][j][:],
                                      kt_d[d, :, j * NJ:(j + 1) * NJ])

            for i in range(RB):
                ncols = (i + 1) * P
                njj = (ncols + NJ - 1) // NJ
                qx = q_pool.tile([P, DC * P], DT, tag="qx")
                for d in range(DC):
                    nc.sync.dma_start(qx[:, d * P:(d + 1) * P], qt_d[i, d])

                etile = e_pool.tile([P, C], DT, tag="E")
                acc = r_pool.tile([P, NKC], F32, tag="acc")
                for jj in range(njj):
                    n = min(NJ, ncols - jj * NJ)
                    ps_s = s_pool.tile([P, NJ], F32, tag="ps_s", name="ps_s")
                    for d in range(DC):
                        nc.tensor.matmul(
                            ps_s[:, :n], qx[:, d * P:(d + 1) * P],
                            kt[d][jj][:, :n],
                            start=(d == 0), stop=(d == DC - 1))
                    if jj == njj - 1:
                        dcol = i * P - jj * NJ
                        nc.vector.tensor_add(ps_s[:, dcol:dcol + P],
                                             ps_s[:, dcol:dcol + P],
                                             cmask[:])
                    nc.scalar.activation(
                        etile[:, jj * NJ:jj * NJ + n], ps_s[:, :n], AF.Exp,
                        scale=scale, accum_out=acc[:, jj:jj + 1])

                rs = r_pool.tile([P, 1], F32, tag="rs")
                nc.vector.reduce_sum(rs[:], acc[:, :njj],
                                     axis=mybir.AxisListType.X)
                rinv = r_pool.tile([P, 1], F32, tag="rinv")
                nc.vector.reciprocal(rinv[:], rs[:])

                ettile = et_pool.tile([P, C], DT, tag="ET")
                for jj in range(njj):
                    n = min(NJ, ncols - jj * NJ)
                    ps_t = t_pool.tile([P, NJ], DT, tag="ps_t")
                    for j in range(n // P):
                        nc.tensor.transpose(
                            ps_t[:, j * P:(j + 1) * P],
                            etile[:, jj * NJ + j * P: jj * NJ + (j + 1) * P],
                            ident_h[:])
                    nc.vector.tensor_copy(ettile[:, jj * NJ:jj * NJ + n],
                                          ps_t[:, :n])

                ps_o = o_pool.tile([P, D], F32, tag="ps_o")
                for dh in range(ND):
                    for j in range(i + 1):
                        nc.tensor.matmul(
                            ps_o[:, dh * NJ:(dh + 1) * NJ],
                            ettile[:, j * P:(j + 1) * P],
                            v_sb[j][:, dh * NJ:(dh + 1) * NJ],
                            start=(j == 0), stop=(j == i))
                outst = os_pool.tile([P, D], F32, tag="outst")
                nc.vector.tensor_scalar_mul(outst[:], ps_o[:], rinv[:])
                nc.sync.dma_start(out_d[i * P:(i + 1) * P, :], outst[:])

    nc.compile()
    return nc


_CACHE = {}


def _built(C=2048, E=1024, D=1024, n_cores=8, loop=1, dt="fp16"):
    key = (C, E, D, n_cores, loop, dt)
    if key not in _CACHE:
        _CACHE[key] = build(C, E, D, n_cores, loop, dt)
    return _CACHE[key]


def _executable(C=2048, E=1024, D=1024, n_cores=8, loop=1, dt="fp16"):
    """Cached jitted SPMD executable for the built Bass module.

    Replicates concourse.bass2jax.run_bass_via_pjrt's multi-core path but
    caches the jit so repeat calls don't retrace, and exposes the pieces
    needed for device-resident benchmarking.
    """
    key = ("exec", C, E, D, n_cores, loop, dt)
    if key in _CACHE:
        return _CACHE[key]
    import jax
    from jax.sharding import Mesh, PartitionSpec
    from jax.experimental.shard_map import shard_map
    from concourse import bass2jax, mybir as _mybir

    nc = _built(C, E, D, n_cores, loop, dt) if isinstance(dt, str) else dt
    bass2jax.install_neuronx_cc_hook()

    partition_name = (nc.partition_id_tensor.name
                      if nc.partition_id_tensor else None)
    in_names, out_names, out_avals, zero_outs = [], [], [], []
    for alloc in nc.m.functions[0].allocations:
        if not isinstance(alloc, _mybir.MemoryLocationSet):
            continue
        name = alloc.memorylocations[0].name
        if alloc.kind == "ExternalInput":
            if name != partition_name:
                in_names.append(name)
        elif alloc.kind == "ExternalOutput":
            out_names.append(name)
            shape = tuple(alloc.tensor_shape)
            dtype = _mybir.dt.np(alloc.dtype)
            out_avals.append(jax.core.ShapedArray(shape, dtype))
            zero_outs.append(np.zeros(shape, dtype))
    n_params = len(in_names)
    all_names = in_names + out_names
    if partition_name is not None:
        all_names = all_names + [partition_name]

    def _body(*args):
        operands = list(args)
        if partition_name is not None:
            operands.append(bass2jax.partition_id_tensor())
        outs = bass2jax._bass_exec_p.bind(
            *operands,
            out_avals=tuple(out_avals),
            in_names=tuple(all_names),
            out_names=tuple(out_names),
            lowering_input_output_aliases=(),
            sim_require_finite=True,
            sim_require_nnan=True,
            nc=nc,
        )
        return tuple(outs)

    devices = jax.devices()[:n_cores]
    mesh = Mesh(np.asarray(devices), ("core",))
    n_outs = len(out_names)
    sharded = jax.jit(
        shard_map(_body, mesh=mesh,
                  in_specs=(PartitionSpec("core"),) * (n_params + n_outs),
                  out_specs=(PartitionSpec("core"),) * n_outs,
                  check_rep=False),
        donate_argnums=tuple(range(n_params, n_params + n_outs)),
        keep_unused=True,
    )
    res = dict(fn=sharded, in_names=in_names, out_names=out_names,
               out_avals=out_avals, zero_outs=zero_outs, mesh=mesh,
               n_cores=n_cores)
    _CACHE[key] = res
    return res


def run(inputs, C=2048, E=1024, D=1024, n_cores=8, dt="fp16"):
    ex = _executable(C, E, D, n_cores, 1, dt)
    B = inputs["x"].shape[0]
    assert B == n_cores
    f = lambda a: np.ascontiguousarray(np.asarray(a, dtype=np.float32))
    shared = {k: f(inputs[k]) for k in ("Qw", "Qb", "Kw", "Kb", "Vw", "Vb")}
    x = f(inputs["x"])
    per_core = [dict(x=x[b], **shared) for b in range(B)]
    concat_in = [
        np.concatenate([per_core[c][n] for c in range(n_cores)], axis=0)
        for n in ex["in_names"]
    ]
    concat_zeros = [
        np.zeros((n_cores * z.shape[0], *z.shape[1:]), z.dtype)
        for z in ex["zero_outs"]
    ]
    out_arrs = ex["fn"](*concat_in, *concat_zeros)
    i = ex["out_names"].index("out")
    out = np.asarray(out_arrs[i]).reshape(n_cores, *ex["out_avals"][i].shape)
    return out


def kernel(**inputs) -> np.ndarray:
    return run(inputs)


# revision 29
# speedup vs baseline: 8456.8647x; 92.6195x over previous
"""Causal single-head attention on 8 trn2 NeuronCores, data-parallel over batch.

Per core (one batch element, C=2048 ctx, E=1024 emb, D=1024 query_dim):
  P_X: transpose x -> xT (PE transpose, fp32), cast to DT, keep resident.
  P_Q: qT = (Qw^T @ x^T) + Qb   -> DRAM scratch (DT), streamed back per row-block.
  P_K: kT = (Kw^T @ x^T) + Kb   -> DRAM scratch (DT), streamed back in phase A.
  P_V: v  = (x @ Vw) + Vb        -> resident SBUF (DT).
  A:   per 128-row query block i: scores = qT_i^T @ kT (causal range only),
       additive -1e9 mask on the diagonal tile, E = exp(scale*scores) with
       fused row-sum on the scalar engine, PE-transpose E, out = sum_j E^T_j @ v_j
       accumulated in PSUM, scaled by 1/rowsum, stored.

DT is the matmul dtype: float16 (default), bfloat16, or float32r.
"""

import os
import sys

for _p in ("/opt/trn_rl_repo", "/root/.axon_site/_ro/trn_rl_repo"):
    if os.path.isdir(_p) and _p not in sys.path:
        sys.path.insert(0, _p)

from contextlib import ExitStack

import numpy as np

import concourse.bass as bass
import concourse.tile as tile
from concourse import bacc, mybir
from concourse.bass_utils import run_bass_kernel_spmd
from concourse.masks import make_causal_mask, make_identity

F32 = mybir.dt.float32
AF = mybir.ActivationFunctionType
DTYPES = {"fp16": mybir.dt.float16, "bf16": mybir.dt.bfloat16,
          "f32r": mybir.dt.float32r}

P = 128


def build(C=2048, E=1024, D=1024, n_cores=8, loop=1, dt="fp16",
          dma_tx=False, dma_te=False):
    DT = DTYPES[dt]
    assert not (dma_tx or dma_te) or DT != mybir.dt.float32r
    CC = 512            # c-chunk width for projection passes
    NJ = 512            # matmul moving-dim width
    NCC = C // CC
    EC = E // P         # contraction chunks for projections
    DC = D // P
    RB = C // P         # number of 128-row blocks
    ND = D // NJ
    scale = float(D) ** -0.5

    nc = bacc.Bacc("TRN2", target_bir_lowering=False, debug=False,
                   num_devices=n_cores)
    x_d = nc.dram_tensor("x", [C, E], F32, kind="ExternalInput").ap()
    qw_d = nc.dram_tensor("Qw", [E, D], F32, kind="ExternalInput").ap()
    qb_d = nc.dram_tensor("Qb", [D], F32, kind="ExternalInput").ap()
    kw_d = nc.dram_tensor("Kw", [E, D], F32, kind="ExternalInput").ap()
    kb_d = nc.dram_tensor("Kb", [D], F32, kind="ExternalInput").ap()
    vw_d = nc.dram_tensor("Vw", [E, D], F32, kind="ExternalInput").ap()
    vb_d = nc.dram_tensor("Vb", [D], F32, kind="ExternalInput").ap()
    out_d = nc.dram_tensor("out", [C, D], F32, kind="ExternalOutput").ap()
    qt_d = nc.dram_tensor("qt_scratch", [RB, DC, P, P], DT, kind="Internal").ap()
    kt_d = nc.dram_tensor("kt_scratch", [DC, P, C], DT, kind="Internal").ap()

    with tile.TileContext(nc) as tc, ExitStack() as ctx:
        if loop > 1:
            ctx.enter_context(tc.For_i(0, loop, 1))
        const_pool = ctx.enter_context(tc.tile_pool(name="const", bufs=1))
        v_pool = ctx.enter_context(tc.tile_pool(name="v", bufs=1))

        # ---- constants
        ident_f = const_pool.tile([P, P], F32, name="ident_f")
        make_identity(nc, ident_f)
        ident_h = const_pool.tile([P, P], DT, name="ident_h")
        nc.vector.tensor_copy(ident_h[:], ident_f[:])
        cmask = const_pool.tile([P, P], F32, name="cmask")
        make_causal_mask(nc, cmask, mask_val=-1e9)
        ones_f = const_pool.tile([1, P], F32, name="ones_f")
        nc.vector.memset(ones_f[:], 1.0)
        ones_h = const_pool.tile([1, P], DT, name="ones_h")
        nc.vector.tensor_copy(ones_h[:], ones_f[:])
        vb_f = const_pool.tile([1, D], F32, name="vb_f")
        nc.sync.dma_start(vb_f[:], vb_d[None, :])
        vb_h = const_pool.tile([1, D], DT, name="vb_h")
        nc.vector.tensor_copy(vb_h[:], vb_f[:])
        qb_t = const_pool.tile([P, DC], F32, name="qb_t")
        nc.sync.dma_start(qb_t[:], qb_d.rearrange("(c p) -> p c", p=P))
        kb_t = const_pool.tile([P, DC], F32, name="kb_t")
        nc.sync.dma_start(kb_t[:], kb_d.rearrange("(c p) -> p c", p=P))

        # ---- resident v
        v_sb = [v_pool.tile([P, D], DT, name=f"v{i}") for i in range(RB)]

        with tc.tile_pool(name="xt", bufs=1) as xt_pool:
            xt = [xt_pool.tile([P, C], DT, name=f"xt{e}") for e in range(EC)]

            # ---- P_X: load + transpose x into xt
            if dma_tx:
                with tc.tile_pool(name="px_in", bufs=4) as xin_pool:
                    for r in range(C // P):
                        xrow = xin_pool.tile([P, E], F32, tag="xrow")
                        nc.sync.dma_start(xrow[:],
                                          x_d[r * P:(r + 1) * P, :])
                        xch = xin_pool.tile([P, E], DT, tag="xch")
                        nc.scalar.copy(xch[:], xrow[:])
                        for e in range(EC):
                            nc.sync.dma_start(
                                xt[e][:, r * P:(r + 1) * P],
                                xch[:, e * P:(e + 1) * P], transpose=True)
            else:
                with tc.tile_pool(name="px_in", bufs=6) as xin_pool, \
                     tc.tile_pool(name="px_ps", bufs=4, space="PSUM") \
                        as pxps_pool:
                    for cc in range(NCC):
                        xrows = []
                        for cs in range(CC // P):
                            xrow = xin_pool.tile([P, E], F32, tag="xrow")
                            nc.sync.dma_start(
                                xrow[:],
                                x_d[cc * CC + cs * P: cc * CC + (cs + 1) * P, :])
                            xrows.append(xrow)
                        for e in range(EC):
                            pst = pxps_pool.tile([P, CC], F32, tag="pst")
                            for cs in range(CC // P):
                                nc.tensor.transpose(
                                    pst[:, cs * P:(cs + 1) * P],
                                    xrows[cs][:, e * P:(e + 1) * P], ident_f[:])
                            nc.scalar.copy(xt[e][:, cc * CC:(cc + 1) * CC],
                                           pst[:])

            with tc.tile_pool(name="w", bufs=1) as w_pool:

                def load_w(w_dram, pname, st_pool):
                    w_sb = []
                    for e in range(EC):
                        wst = st_pool.tile([P, D], F32, tag="wst",
                                           name=f"{pname}st{e}")
                        nc.sync.dma_start(wst[:], w_dram[e * P:(e + 1) * P, :])
                        wt = w_pool.tile([P, D], DT, tag=f"w{e}",
                                         name=f"{pname}{e}")
                        nc.vector.tensor_copy(wt[:], wst[:])
                        w_sb.append(wt)
                    return w_sb

                # ---- P_Q / P_K: qT/kT tiles -> DRAM scratch.
                # Consecutive matmuls rotate the stationary operand
                # (same-stationary back-to-back measures 2x slower).
                def proj_t(w_sb, bias_t, store):
                    with tc.tile_pool(name="pp_ps", bufs=6, space="PSUM") \
                            as ps_pool, \
                         tc.tile_pool(name="pp_st", bufs=3) as st_pool:
                        for cc in range(NCC):
                            for dc in range(DC):
                                ps = ps_pool.tile([P, CC], F32, tag="ps")
                                for e in range(EC):
                                    nc.tensor.matmul(
                                        ps[:],
                                        w_sb[e][:, dc * P:(dc + 1) * P],
                                        xt[e][:, cc * CC:(cc + 1) * CC],
                                        start=(e == 0), stop=(e == EC - 1))
                                pst = st_pool.tile([P, CC], DT, tag="pst")
                                nc.scalar.activation(
                                    pst[:], ps[:], AF.Identity,
                                    bias=bias_t[:, dc:dc + 1])
                                store(dc, cc, pst)

                with tc.tile_pool(name="pq_w", bufs=2) as wstp:
                    qw_sb = load_w(qw_d, "qw", wstp)

                    def store_q(dc, cc, pst):
                        for rb in range(CC // P):
                            nc.sync.dma_start(
                                qt_d[cc * (CC // P) + rb, dc],
                                pst[:, rb * P:(rb + 1) * P])
                    proj_t(qw_sb, qb_t, store_q)

                with tc.tile_pool(name="pk_w", bufs=2) as wstp:
                    kw_sb = load_w(kw_d, "kw", wstp)

                    def store_k(dc, cc, pst):
                        nc.sync.dma_start(
                            kt_d[dc, :, cc * CC:(cc + 1) * CC], pst[:])
                    proj_t(kw_sb, kb_t, store_k)

                # ---- P_V: v resident (natural layout)
                with tc.tile_pool(name="pv_ps", bufs=6, space="PSUM") \
                        as ps_pool, \
                     tc.tile_pool(name="pv_w", bufs=2) as wstp:
                    vw_sb = load_w(vw_d, "vw", wstp)
                    for ct in range(RB):
                        for dh in range(ND):
                            ps = ps_pool.tile([P, NJ], F32, tag="ps")
                            for e in range(EC):
                                nc.tensor.matmul(
                                    ps[:], xt[e][:, ct * P:(ct + 1) * P],
                                    vw_sb[e][:, dh * NJ:(dh + 1) * NJ],
                                    start=(e == 0), stop=False)
                            nc.tensor.matmul(ps[:], ones_h[:],
                                             vb_h[:, dh * NJ:(dh + 1) * NJ],
                                             start=False, stop=True)
                            nc.scalar.copy(v_sb[ct][:, dh * NJ:(dh + 1) * NJ],
                                           ps[:])

        # ---- Phase A: causal attention per row-block
        with tc.tile_pool(name="kt", bufs=1) as kt_pool, \
             tc.tile_pool(name="q", bufs=2) as q_pool, \
             tc.tile_pool(name="e", bufs=2) as e_pool, \
             tc.tile_pool(name="et", bufs=2) as et_pool, \
             tc.tile_pool(name="r", bufs=2) as r_pool, \
             tc.tile_pool(name="os", bufs=2) as os_pool, \
             tc.tile_pool(name="a_s", bufs=2, space="PSUM") as s_pool, \
             tc.tile_pool(name="a_t", bufs=2, space="PSUM") as t_pool, \
             tc.tile_pool(name="a_o", bufs=2, space="PSUM") as o_pool:
            NKC = C // NJ
            kt = [[kt_pool.tile([P, NJ], DT, name=f"kt{d}_{j}")
                   for j in range(NKC)] for d in range(DC)]
            for j in range(NKC):
                for d in range(DC):
                    nc.sync.dma_start(kt[d][j][:],
                                      kt_d[d, :, j * NJ:(j + 1) * NJ])

            for i in range(RB):
                ncols = (i + 1) * P
                njj = (ncols + NJ - 1) // NJ
                qx = q_pool.tile([P, DC * P], DT, tag="qx")
                for d in range(DC):
                    nc.sync.dma_start(qx[:, d * P:(d + 1) * P], qt_d[i, d])

                etile = e_pool.tile([P, C], DT, tag="E")
                acc = r_pool.tile([P, NKC], F32, tag="acc")
                for jj in range(njj):
                    n = min(NJ, ncols - jj * NJ)
                    ps_s = s_pool.tile([P, NJ], F32, tag="ps_s", name="ps_s")
                    for d in range(DC):
                        nc.tensor.matmul(
                            ps_s[:, :n], qx[:, d * P:(d + 1) * P],
                            kt[d][jj][:, :n],
                            start=(d == 0), stop=(d == DC - 1))
                    if jj == njj - 1:
                        dcol = i * P - jj * NJ
                        nc.vector.tensor_add(ps_s[:, dcol:dcol + P],
                                             ps_s[:, dcol:dcol + P],
                                             cmask[:])
                    nc.scalar.activation(
                        etile[:, jj * NJ:jj * NJ + n], ps_s[:, :n], AF.Exp,
                        scale=scale, accum_out=acc[:, jj:jj + 1])

                rs = r_pool.tile([P, 1], F32, tag="rs")
                nc.vector.reduce_sum(rs[:], acc[:, :njj],
                                     axis=mybir.AxisListType.X)
                rinv = r_pool.tile([P, 1], F32, tag="rinv")
                nc.vector.reciprocal(rinv[:], rs[:])

                ettile = et_pool.tile([P, C], DT, tag="ET")
                if dma_te:
                    for j in range(i + 1):
                        nc.sync.dma_start(ettile[:, j * P:(j + 1) * P],
                                          etile[:, j * P:(j + 1) * P],
                                          transpose=True)
                else:
                    for jj in range(njj):
                        n = min(NJ, ncols - jj * NJ)
                        ps_t = t_pool.tile([P, NJ], DT, tag="ps_t")
                        for j in range(n // P):
                            nc.tensor.transpose(
                                ps_t[:, j * P:(j + 1) * P],
                                etile[:, jj * NJ + j * P: jj * NJ + (j + 1) * P],
                                ident_h[:])
                        nc.vector.tensor_copy(ettile[:, jj * NJ:jj * NJ + n],
                                              ps_t[:, :n])

                ps_o = o_pool.tile([P, D], F32, tag="ps_o")
                for dh in range(ND):
                    for j in range(i + 1):
                        nc.tensor.matmul(
                            ps_o[:, dh * NJ:(dh + 1) * NJ],
                            ettile[:, j * P:(j + 1) * P],
                            v_sb[j][:, dh * NJ:(dh + 1) * NJ],
                            start=(j == 0), stop=(j == i))
                outst = os_pool.tile([P, D], F32, tag="outst")
                nc.vector.tensor_scalar_mul(outst[:], ps_o[:], rinv[:])
                nc.sync.dma_start(out_d[i * P:(i + 1) * P, :], outst[:])

    nc.compile()
    return nc


_CACHE = {}


def _built(C=2048, E=1024, D=1024, n_cores=8, loop=1, dt="fp16",
           dma_tx=False, dma_te=False):
    key = (C, E, D, n_cores, loop, dt, dma_tx, dma_te)
    if key not in _CACHE:
        _CACHE[key] = build(C, E, D, n_cores, loop, dt, dma_tx, dma_te)
    return _CACHE[key]


def _executable(C=2048, E=1024, D=1024, n_cores=8, loop=1, dt="fp16",
                dma_tx=False, dma_te=False):
    """Cached jitted SPMD executable for the built Bass module.

    Replicates concourse.bass2jax.run_bass_via_pjrt's multi-core path but
    caches the jit so repeat calls don't retrace, and exposes the pieces
    needed for device-resident benchmarking.
    """
    key = ("exec", C, E, D, n_cores, loop, dt, dma_tx, dma_te)
    if key in _CACHE:
        return _CACHE[key]
    import jax
    from jax.sharding import Mesh, PartitionSpec
    from jax.experimental.shard_map import shard_map
    from concourse import bass2jax, mybir as _mybir

    nc = _built(C, E, D, n_cores, loop, dt, dma_tx, dma_te)
    bass2jax.install_neuronx_cc_hook()

    partition_name = (nc.partition_id_tensor.name
                      if nc.partition_id_tensor else None)
    in_names, out_names, out_avals, zero_outs = [], [], [], []
    for alloc in nc.m.functions[0].allocations:
        if not isinstance(alloc, _mybir.MemoryLocationSet):
            continue
        name = alloc.memorylocations[0].name
        if alloc.kind == "ExternalInput":
            if name != partition_name:
                in_names.append(name)
        elif alloc.kind == "ExternalOutput":
            out_names.append(name)
            shape = tuple(alloc.tensor_shape)
            dtype = _mybir.dt.np(alloc.dtype)
            out_avals.append(jax.core.ShapedArray(shape, dtype))
            zero_outs.append(np.zeros(shape, dtype))
    n_params = len(in_names)
    all_names = in_names + out_names
    if partition_name is not None:
        all_names = all_names + [partition_name]

    def _body(*args):
        operands = list(args)
        if partition_name is not None:
            operands.append(bass2jax.partition_id_tensor())
        outs = bass2jax._bass_exec_p.bind(
            *operands,
            out_avals=tuple(out_avals),
            in_names=tuple(all_names),
            out_names=tuple(out_names),
            lowering_input_output_aliases=(),
            sim_require_finite=True,
            sim_require_nnan=True,
            nc=nc,
        )
        return tuple(outs)

    devices = jax.devices()[:n_cores]
    mesh = Mesh(np.asarray(devices), ("core",))
    n_outs = len(out_names)
    sharded = jax.jit(
        shard_map(_body, mesh=mesh,
                  in_specs=(PartitionSpec("core"),) * (n_params + n_outs),
                  out_specs=(PartitionSpec("core"),) * n_outs,
                  check_rep=False),
        donate_argnums=tuple(range(n_params, n_params + n_outs)),
        keep_unused=True,
    )
    res = dict(fn=sharded, in_names=in_names, out_names=out_names,
               out_avals=out_avals, zero_outs=zero_outs, mesh=mesh,
               n_cores=n_cores)
    _CACHE[key] = res
    return res


def run(inputs, C=2048, E=1024, D=1024, n_cores=8, dt="fp16"):
    ex = _executable(C, E, D, n_cores, 1, dt)
    B = inputs["x"].shape[0]
    assert B == n_cores
    f = lambda a: np.ascontiguousarray(np.asarray(a, dtype=np.float32))
    shared = {k: f(inputs[k]) for k in ("Qw", "Qb", "Kw", "Kb", "Vw", "Vb")}
    x = f(inputs["x"])
    per_core = [dict(x=x[b], **shared) for b in range(B)]
    concat_in = [
        np.concatenate([per_core[c][n] for c in range(n_cores)], axis=0)
        for n in ex["in_names"]
    ]
    concat_zeros = [
        np.zeros((n_cores * z.shape[0], *z.shape[1:]), z.dtype)
        for z in ex["zero_outs"]
    ]
    out_arrs = ex["fn"](*concat_in, *concat_zeros)
    i = ex["out_names"].index("out")
    out = np.asarray(out_arrs[i]).reshape(n_cores, *ex["out_avals"][i].shape)
    return out


def kernel(**inputs) -> np.ndarray:
    return run(inputs)


# revision 34
# speedup vs baseline: 9255.2157x; 1.0944x over previous
"""Causal single-head attention on 8 trn2 NeuronCores, data-parallel over batch.

Per core (one batch element, C=2048 ctx, E=1024 emb, D=1024 query_dim):
  P_X: transpose x -> xT (PE transpose, fp32), cast to DT, resident.
  P_Q: qT = (Qw^T @ x^T) + Qb   -> resident SBUF (DT), [d-chunk][128, C].
  P_K: kT = (Kw^T @ x^T) + Kb   -> resident SBUF (DT), same layout.
  P_V: v  = (x @ Vw) + Vb        -> resident SBUF (DT), natural layout.
  A:   per 128-row query block i: scores = qT_i^T @ kT (causal range only),
       additive -1e9 mask on the diagonal tile, E = exp(scale*scores) with
       fused row-sum on the scalar engine, PE-transpose E, out = sum_j E^T_j @ v_j
       accumulated in PSUM, scaled by 1/rowsum, stored.

DT is the matmul dtype: float16 (default), bfloat16. (float32r works only in
the DRAM-scratch variant of this file's history; fp16 keeps everything
resident and is ~2x more accurate than bf16 at the same speed.)
"""

import os
import sys

for _p in ("/opt/trn_rl_repo", "/root/.axon_site/_ro/trn_rl_repo"):
    if os.path.isdir(_p) and _p not in sys.path:
        sys.path.insert(0, _p)

from contextlib import ExitStack

import numpy as np

import concourse.bass as bass
import concourse.tile as tile
from concourse import bacc, mybir
from concourse.masks import make_causal_mask, make_identity

F32 = mybir.dt.float32
AF = mybir.ActivationFunctionType
DTYPES = {"fp16": mybir.dt.float16, "bf16": mybir.dt.bfloat16}

P = 128


def build(C=2048, E=1024, D=1024, n_cores=8, loop=1, dt="fp16"):
    DT = DTYPES[dt]
    CC = 512            # c-chunk width for projection passes
    NJ = 512            # matmul moving-dim width
    NCC = C // CC
    EC = E // P         # contraction chunks for projections
    DC = D // P
    RB = C // P         # number of 128-row blocks
    ND = D // NJ
    scale = float(D) ** -0.5

    nc = bacc.Bacc("TRN2", target_bir_lowering=False, debug=False,
                   num_devices=n_cores)
    x_d = nc.dram_tensor("x", [C, E], F32, kind="ExternalInput").ap()
    qw_d = nc.dram_tensor("Qw", [E, D], F32, kind="ExternalInput").ap()
    qb_d = nc.dram_tensor("Qb", [D], F32, kind="ExternalInput").ap()
    kw_d = nc.dram_tensor("Kw", [E, D], F32, kind="ExternalInput").ap()
    kb_d = nc.dram_tensor("Kb", [D], F32, kind="ExternalInput").ap()
    vw_d = nc.dram_tensor("Vw", [E, D], F32, kind="ExternalInput").ap()
    vb_d = nc.dram_tensor("Vb", [D], F32, kind="ExternalInput").ap()
    out_d = nc.dram_tensor("out", [C, D], F32, kind="ExternalOutput").ap()

    with tile.TileContext(nc) as tc, ExitStack() as ctx:
        if loop > 1:
            ctx.enter_context(tc.For_i(0, loop, 1))
        const_pool = ctx.enter_context(tc.tile_pool(name="const", bufs=1))
        v_pool = ctx.enter_context(tc.tile_pool(name="v", bufs=1))
        qt_pool = ctx.enter_context(tc.tile_pool(name="qt", bufs=1))
        kt_pool = ctx.enter_context(tc.tile_pool(name="kt", bufs=1))

        # ---- constants
        ident_f = const_pool.tile([P, P], F32, name="ident_f")
        make_identity(nc, ident_f)
        ident_h = const_pool.tile([P, P], DT, name="ident_h")
        nc.vector.tensor_copy(ident_h[:], ident_f[:])
        cmask = const_pool.tile([P, P], F32, name="cmask")
        make_causal_mask(nc, cmask, mask_val=-1e9)
        ones_f = const_pool.tile([1, P], F32, name="ones_f")
        nc.vector.memset(ones_f[:], 1.0)
        ones_h = const_pool.tile([1, P], DT, name="ones_h")
        nc.vector.tensor_copy(ones_h[:], ones_f[:])
        vb_f = const_pool.tile([1, D], F32, name="vb_f")
        nc.sync.dma_start(vb_f[:], vb_d[None, :])
        vb_h = const_pool.tile([1, D], DT, name="vb_h")
        nc.vector.tensor_copy(vb_h[:], vb_f[:])
        qb_t = const_pool.tile([P, DC], F32, name="qb_t")
        nc.sync.dma_start(qb_t[:], qb_d.rearrange("(c p) -> p c", p=P))
        kb_t = const_pool.tile([P, DC], F32, name="kb_t")
        nc.sync.dma_start(kb_t[:], kb_d.rearrange("(c p) -> p c", p=P))

        # ---- resident tensors
        v_sb = [v_pool.tile([P, D], DT, name=f"v{i}") for i in range(RB)]
        qt_sb = [qt_pool.tile([P, C], DT, name=f"qt{d}") for d in range(DC)]
        kt_sb = [kt_pool.tile([P, C], DT, name=f"kt{d}") for d in range(DC)]

        with tc.tile_pool(name="xt", bufs=1) as xt_pool:
            xt = [xt_pool.tile([P, C], DT, name=f"xt{e}") for e in range(EC)]

            # ---- P_X: load x (SWDGE dma casts fp32->DT), transpose into xt
            with tc.tile_pool(name="px_in", bufs=6) as xin_pool, \
                 tc.tile_pool(name="px_ps", bufs=3, space="PSUM") as pxps_pool:
                for cc in range(NCC):
                    xrows = []
                    for cs in range(CC // P):
                        xrow = xin_pool.tile([P, E], DT, tag="xrow")
                        nc.gpsimd.dma_start(
                            xrow[:],
                            x_d[cc * CC + cs * P: cc * CC + (cs + 1) * P, :])
                        xrows.append(xrow)
                    for e in range(EC):
                        pst = pxps_pool.tile([P, CC], DT, tag="pst")
                        for cs in range(CC // P):
                            nc.tensor.transpose(
                                pst[:, cs * P:(cs + 1) * P],
                                xrows[cs][:, e * P:(e + 1) * P], ident_h[:])
                        nc.scalar.copy(xt[e][:, cc * CC:(cc + 1) * CC], pst[:])

            with tc.tile_pool(name="w", bufs=1) as w_pool:

                def load_w(w_dram, pname, st_pool):
                    w_sb = []
                    for e in range(EC):
                        wst = st_pool.tile([P, D], F32, tag="wst",
                                           name=f"{pname}st{e}")
                        nc.sync.dma_start(wst[:], w_dram[e * P:(e + 1) * P, :])
                        wt = w_pool.tile([P, D], DT, tag=f"w{e}",
                                         name=f"{pname}{e}")
                        nc.gpsimd.tensor_copy(wt[:], wst[:])
                        w_sb.append(wt)
                    return w_sb

                # ---- P_Q / P_K: write projections straight into resident
                # transposed tiles via the scalar engine (bias fused).
                def proj_t(w_sb, bias_t, dest):
                    with tc.tile_pool(name="pp_ps", bufs=4, space="PSUM") \
                            as ps_pool:
                        for cc in range(NCC):
                            for dc in range(DC):
                                ps = ps_pool.tile([P, CC], F32, tag="ps")
                                for e in range(EC):
                                    nc.tensor.matmul(
                                        ps[:],
                                        w_sb[e][:, dc * P:(dc + 1) * P],
                                        xt[e][:, cc * CC:(cc + 1) * CC],
                                        start=(e == 0), stop=(e == EC - 1))
                                nc.scalar.activation(
                                    dest[dc][:, cc * CC:(cc + 1) * CC], ps[:],
                                    AF.Identity, bias=bias_t[:, dc:dc + 1])

                with tc.tile_pool(name="pq_w", bufs=2) as wstp:
                    qw_sb = load_w(qw_d, "qw", wstp)
                    proj_t(qw_sb, qb_t, qt_sb)

                with tc.tile_pool(name="pk_w", bufs=2) as wstp:
                    kw_sb = load_w(kw_d, "kw", wstp)
                    proj_t(kw_sb, kb_t, kt_sb)

                # ---- P_V: v resident (natural layout)
                with tc.tile_pool(name="pv_ps", bufs=3, space="PSUM") \
                        as ps_pool, \
                     tc.tile_pool(name="pv_w", bufs=2) as wstp:
                    vw_sb = load_w(vw_d, "vw", wstp)
                    for ct in range(RB):
                        for dh in range(ND):
                            ps = ps_pool.tile([P, NJ], F32, tag="ps")
                            for e in range(EC):
                                nc.tensor.matmul(
                                    ps[:], xt[e][:, ct * P:(ct + 1) * P],
                                    vw_sb[e][:, dh * NJ:(dh + 1) * NJ],
                                    start=(e == 0), stop=False)
                            nc.tensor.matmul(ps[:], ones_h[:],
                                             vb_h[:, dh * NJ:(dh + 1) * NJ],
                                             start=False, stop=True)
                            nc.vector.tensor_copy(
                                v_sb[ct][:, dh * NJ:(dh + 1) * NJ], ps[:])

        # ---- Phase A: causal attention per row-block
        with tc.tile_pool(name="e", bufs=2) as e_pool, \
             tc.tile_pool(name="et", bufs=2) as et_pool, \
             tc.tile_pool(name="r", bufs=2) as r_pool, \
             tc.tile_pool(name="os", bufs=2) as os_pool, \
             tc.tile_pool(name="a_s", bufs=2, space="PSUM") as s_pool, \
             tc.tile_pool(name="a_t", bufs=2, space="PSUM") as t_pool, \
             tc.tile_pool(name="a_o", bufs=2, space="PSUM") as o_pool:
            NKC = C // NJ
            for i in range(RB):
                ncols = (i + 1) * P
                njj = (ncols + NJ - 1) // NJ

                etile = e_pool.tile([P, C], DT, tag="E")
                acc = r_pool.tile([P, NKC], F32, tag="acc")
                for jj in range(njj):
                    n = min(NJ, ncols - jj * NJ)
                    ps_s = s_pool.tile([P, NJ], F32, tag="ps_s", name="ps_s")
                    for d in range(DC):
                        nc.tensor.matmul(
                            ps_s[:, :n],
                            qt_sb[d][:, i * P:(i + 1) * P],
                            kt_sb[d][:, jj * NJ:jj * NJ + n],
                            start=(d == 0), stop=(d == DC - 1))
                    if jj == njj - 1:
                        dcol = i * P - jj * NJ
                        nc.vector.tensor_add(ps_s[:, dcol:dcol + P],
                                             ps_s[:, dcol:dcol + P],
                                             cmask[:])
                    nc.scalar.activation(
                        etile[:, jj * NJ:jj * NJ + n], ps_s[:, :n], AF.Exp,
                        scale=scale, accum_out=acc[:, jj:jj + 1])

                rs = r_pool.tile([P, 1], F32, tag="rs")
                nc.vector.reduce_sum(rs[:], acc[:, :njj],
                                     axis=mybir.AxisListType.X)
                rinv = r_pool.tile([P, 1], F32, tag="rinv")
                nc.vector.reciprocal(rinv[:], rs[:])

                ettile = et_pool.tile([P, C], DT, tag="ET")
                for jj in range(njj):
                    n = min(NJ, ncols - jj * NJ)
                    ps_t = t_pool.tile([P, NJ], DT, tag="ps_t")
                    for j in range(n // P):
                        nc.tensor.transpose(
                            ps_t[:, j * P:(j + 1) * P],
                            etile[:, jj * NJ + j * P: jj * NJ + (j + 1) * P],
                            ident_h[:])
                    nc.vector.tensor_copy(ettile[:, jj * NJ:jj * NJ + n],
                                          ps_t[:, :n])

                ps_o = o_pool.tile([P, D], F32, tag="ps_o")
                for dh in range(ND):
                    for j in range(i + 1):
                        nc.tensor.matmul(
                            ps_o[:, dh * NJ:(dh + 1) * NJ],
                            ettile[:, j * P:(j + 1) * P],
                            v_sb[j][:, dh * NJ:(dh + 1) * NJ],
                            start=(j == 0), stop=(j == i))
                outst = os_pool.tile([P, D], F32, tag="outst")
                nc.vector.tensor_scalar_mul(outst[:], ps_o[:], rinv[:])
                nc.sync.dma_start(out_d[i * P:(i + 1) * P, :], outst[:])

    nc.compile()
    return nc


_CACHE = {}


def _built(C=2048, E=1024, D=1024, n_cores=8, loop=1, dt="fp16"):
    key = (C, E, D, n_cores, loop, dt)
    if key not in _CACHE:
        _CACHE[key] = build(C, E, D, n_cores, loop, dt)
    return _CACHE[key]


def _executable(C=2048, E=1024, D=1024, n_cores=8, loop=1, dt="fp16"):
    """Cached jitted SPMD executable for the built Bass module.

    Replicates concourse.bass2jax.run_bass_via_pjrt's multi-core path but
    caches the jit so repeat calls don't retrace, and exposes the pieces
    needed for device-resident benchmarking.
    """
    key = ("exec", C, E, D, n_cores, loop, dt)
    if key in _CACHE:
        return _CACHE[key]
    import jax
    from jax.sharding import Mesh, PartitionSpec
    from jax.experimental.shard_map import shard_map
    from concourse import bass2jax, mybir as _mybir

    nc = _built(C, E, D, n_cores, loop, dt)
    bass2jax.install_neuronx_cc_hook()

    partition_name = (nc.partition_id_tensor.name
                      if nc.partition_id_tensor else None)
    in_names, out_names, out_avals, zero_outs = [], [], [], []
    for alloc in nc.m.functions[0].allocations:
        if not isinstance(alloc, _mybir.MemoryLocationSet):
            continue
        name = alloc.memorylocations[0].name
        if alloc.kind == "ExternalInput":
            if name != partition_name:
                in_names.append(name)
        elif alloc.kind == "ExternalOutput":
            out_names.append(name)
            shape = tuple(alloc.tensor_shape)
            dtype = _mybir.dt.np(alloc.dtype)
            out_avals.append(jax.core.ShapedArray(shape, dtype))
            zero_outs.append(np.zeros(shape, dtype))
    n_params = len(in_names)
    all_names = in_names + out_names
    if partition_name is not None:
        all_names = all_names + [partition_name]

    def _body(*args):
        operands = list(args)
        if partition_name is not None:
            operands.append(bass2jax.partition_id_tensor())
        outs = bass2jax._bass_exec_p.bind(
            *operands,
            out_avals=tuple(out_avals),
            in_names=tuple(all_names),
            out_names=tuple(out_names),
            lowering_input_output_aliases=(),
            sim_require_finite=True,
            sim_require_nnan=True,
            nc=nc,
        )
        return tuple(outs)

    devices = jax.devices()[:n_cores]
    mesh = Mesh(np.asarray(devices), ("core",))
    n_outs = len(out_names)
    sharded = jax.jit(
        shard_map(_body, mesh=mesh,
                  in_specs=(PartitionSpec("core"),) * (n_params + n_outs),
                  out_specs=(PartitionSpec("core"),) * n_outs,
                  check_rep=False),
        donate_argnums=tuple(range(n_params, n_params + n_outs)),
        keep_unused=True,
    )
    res = dict(fn=sharded, in_names=in_names, out_names=out_names,
               out_avals=out_avals, zero_outs=zero_outs, mesh=mesh,
               n_cores=n_cores)
    _CACHE[key] = res
    return res


def run(inputs, C=2048, E=1024, D=1024, n_cores=8, dt="fp16"):
    ex = _executable(C, E, D, n_cores, 1, dt)
    B = inputs["x"].shape[0]
    assert B == n_cores
    f = lambda a: np.ascontiguousarray(np.asarray(a, dtype=np.float32))
    shared = {k: f(inputs[k]) for k in ("Qw", "Qb", "Kw", "Kb", "Vw", "Vb")}
    x = f(inputs["x"])
    per_core = [dict(x=x[b], **shared) for b in range(B)]
    concat_in = [
        np.concatenate([per_core[c][n] for c in range(n_cores)], axis=0)
        for n in ex["in_names"]
    ]
    concat_zeros = [
        np.zeros((n_cores * z.shape[0], *z.shape[1:]), z.dtype)
        for z in ex["zero_outs"]
    ]
    out_arrs = ex["fn"](*concat_in, *concat_zeros)
    i = ex["out_names"].index("out")
    out = np.asarray(out_arrs[i]).reshape(n_cores, *ex["out_avals"][i].shape)
    return out


def kernel(**inputs) -> np.ndarray:
    return run(inputs)


# revision 37
# speedup vs baseline: 10014.7440x; 1.0821x over previous
"""Causal single-head attention on 8 trn2 NeuronCores, data-parallel over batch.

Per core (one batch element, C=2048 ctx, E=1024 emb, D=1024 query_dim):
  P_X: transpose x -> xT (PE transpose, fp32), cast to DT, resident.
  P_Q: qT = (Qw^T @ x^T) + Qb   -> resident SBUF (DT), [d-chunk][128, C].
  P_K: kT = (Kw^T @ x^T) + Kb   -> resident SBUF (DT), same layout.
  P_V: v  = (x @ Vw) + Vb        -> resident SBUF (DT), natural layout.
  A:   per 128-row query block i: scores = qT_i^T @ kT (causal range only),
       additive -1e9 mask on the diagonal tile, E = exp(scale*scores) with
       fused row-sum on the scalar engine, PE-transpose E, out = sum_j E^T_j @ v_j
       accumulated in PSUM, scaled by 1/rowsum, stored.

DT is the matmul dtype: float16 (default), bfloat16. (float32r works only in
the DRAM-scratch variant of this file's history; fp16 keeps everything
resident and is ~2x more accurate than bf16 at the same speed.)
"""

import os
import sys

for _p in ("/opt/trn_rl_repo", "/root/.axon_site/_ro/trn_rl_repo"):
    if os.path.isdir(_p) and _p not in sys.path:
        sys.path.insert(0, _p)

from contextlib import ExitStack

import numpy as np

import concourse.bass as bass
import concourse.tile as tile
from concourse import bacc, mybir
from concourse.masks import make_causal_mask, make_identity

F32 = mybir.dt.float32
AF = mybir.ActivationFunctionType
DTYPES = {"fp16": mybir.dt.float16, "bf16": mybir.dt.bfloat16}

P = 128


def build(C=2048, E=1024, D=1024, n_cores=8, loop=1, dt="fp16"):
    DT = DTYPES[dt]
    CC = 512            # c-chunk width for projection passes
    NJ = 512            # matmul moving-dim width
    NCC = C // CC
    EC = E // P         # contraction chunks for projections
    DC = D // P
    RB = C // P         # number of 128-row blocks
    ND = D // NJ
    scale = float(D) ** -0.5

    nc = bacc.Bacc("TRN2", target_bir_lowering=False, debug=False,
                   num_devices=n_cores)
    x_d = nc.dram_tensor("x", [C, E], F32, kind="ExternalInput").ap()
    qw_d = nc.dram_tensor("Qw", [E, D], F32, kind="ExternalInput").ap()
    qb_d = nc.dram_tensor("Qb", [D], F32, kind="ExternalInput").ap()
    kw_d = nc.dram_tensor("Kw", [E, D], F32, kind="ExternalInput").ap()
    kb_d = nc.dram_tensor("Kb", [D], F32, kind="ExternalInput").ap()
    vw_d = nc.dram_tensor("Vw", [E, D], F32, kind="ExternalInput").ap()
    vb_d = nc.dram_tensor("Vb", [D], F32, kind="ExternalInput").ap()
    out_d = nc.dram_tensor("out", [C, D], F32, kind="ExternalOutput").ap()

    with tile.TileContext(nc) as tc, ExitStack() as ctx:
        if loop > 1:
            ctx.enter_context(tc.For_i(0, loop, 1))
        const_pool = ctx.enter_context(tc.tile_pool(name="const", bufs=1))
        v_pool = ctx.enter_context(tc.tile_pool(name="v", bufs=1))
        qt_pool = ctx.enter_context(tc.tile_pool(name="qt", bufs=1))
        kt_pool = ctx.enter_context(tc.tile_pool(name="kt", bufs=1))

        # ---- constants
        ident_f = const_pool.tile([P, P], F32, name="ident_f")
        make_identity(nc, ident_f)
        ident_h = const_pool.tile([P, P], DT, name="ident_h")
        nc.vector.tensor_copy(ident_h[:], ident_f[:])
        cmask = const_pool.tile([P, P], F32, name="cmask")
        make_causal_mask(nc, cmask, mask_val=-1e9)
        ones_f = const_pool.tile([1, P], F32, name="ones_f")
        nc.vector.memset(ones_f[:], 1.0)
        ones_h = const_pool.tile([1, P], DT, name="ones_h")
        nc.vector.tensor_copy(ones_h[:], ones_f[:])
        vb_f = const_pool.tile([1, D], F32, name="vb_f")
        nc.sync.dma_start(vb_f[:], vb_d[None, :])
        vb_h = const_pool.tile([1, D], DT, name="vb_h")
        nc.vector.tensor_copy(vb_h[:], vb_f[:])
        qb_t = const_pool.tile([P, DC], F32, name="qb_t")
        nc.sync.dma_start(qb_t[:], qb_d.rearrange("(c p) -> p c", p=P))
        kb_t = const_pool.tile([P, DC], F32, name="kb_t")
        nc.sync.dma_start(kb_t[:], kb_d.rearrange("(c p) -> p c", p=P))

        # ---- resident tensors
        v_sb = [v_pool.tile([P, D], DT, name=f"v{i}") for i in range(RB)]
        qt_sb = [qt_pool.tile([P, C], DT, name=f"qt{d}") for d in range(DC)]
        kt_sb = [kt_pool.tile([P, C], DT, name=f"kt{d}") for d in range(DC)]

        with tc.tile_pool(name="xt", bufs=1) as xt_pool:
            xt = [xt_pool.tile([P, C], DT, name=f"xt{e}") for e in range(EC)]

            # ---- P_X: load x (SWDGE dma casts fp32->DT), transpose into xt
            with tc.tile_pool(name="px_in", bufs=6) as xin_pool, \
                 tc.tile_pool(name="px_ps", bufs=4, space="PSUM") as pxps_pool:
                for cc in range(NCC):
                    xrows = []
                    for cs in range(CC // P):
                        xrow = xin_pool.tile([P, E], F32, tag="xrow")
                        nc.sync.dma_start(
                            xrow[:],
                            x_d[cc * CC + cs * P: cc * CC + (cs + 1) * P, :])
                        xrow16 = xin_pool.tile([P, E], DT, tag="xrow16")
                        nc.scalar.copy(xrow16[:], xrow[:])
                        xrows.append(xrow16)
                    for e in range(EC):
                        pst = pxps_pool.tile([P, CC], DT, tag="pst")
                        for cs in range(CC // P):
                            nc.tensor.transpose(
                                pst[:, cs * P:(cs + 1) * P],
                                xrows[cs][:, e * P:(e + 1) * P], ident_h[:])
                        nc.scalar.copy(xt[e][:, cc * CC:(cc + 1) * CC], pst[:])

            with tc.tile_pool(name="w", bufs=1) as w_pool:

                def load_w(w_dram, pname, st_pool):
                    w_sb = []
                    for e in range(EC):
                        wst = st_pool.tile([P, D], F32, tag="wst",
                                           name=f"{pname}st{e}")
                        nc.sync.dma_start(wst[:], w_dram[e * P:(e + 1) * P, :])
                        wt = w_pool.tile([P, D], DT, tag=f"w{e}",
                                         name=f"{pname}{e}")
                        nc.vector.tensor_copy(wt[:], wst[:])
                        w_sb.append(wt)
                    return w_sb

                # ---- P_Q / P_K: write projections straight into resident
                # transposed tiles via the scalar engine (bias fused).
                def proj_t(w_sb, bias_t, dest):
                    with tc.tile_pool(name="pp_ps", bufs=6, space="PSUM") \
                            as ps_pool:
                        for cc in range(NCC):
                            for dc in range(DC):
                                ps = ps_pool.tile([P, CC], F32, tag="ps")
                                for e in range(EC):
                                    nc.tensor.matmul(
                                        ps[:],
                                        w_sb[e][:, dc * P:(dc + 1) * P],
                                        xt[e][:, cc * CC:(cc + 1) * CC],
                                        start=(e == 0), stop=(e == EC - 1))
                                nc.scalar.activation(
                                    dest[dc][:, cc * CC:(cc + 1) * CC], ps[:],
                                    AF.Identity, bias=bias_t[:, dc:dc + 1])

                with tc.tile_pool(name="pq_w", bufs=2) as wstp:
                    qw_sb = load_w(qw_d, "qw", wstp)
                    proj_t(qw_sb, qb_t, qt_sb)

                with tc.tile_pool(name="pk_w", bufs=2) as wstp:
                    kw_sb = load_w(kw_d, "kw", wstp)
                    proj_t(kw_sb, kb_t, kt_sb)

                # ---- P_V: v resident (natural layout)
                with tc.tile_pool(name="pv_ps", bufs=6, space="PSUM") \
                        as ps_pool, \
                     tc.tile_pool(name="pv_w", bufs=2) as wstp:
                    vw_sb = load_w(vw_d, "vw", wstp)
                    for ct in range(RB):
                        for dh in range(ND):
                            ps = ps_pool.tile([P, NJ], F32, tag="ps")
                            for e in range(EC):
                                nc.tensor.matmul(
                                    ps[:], xt[e][:, ct * P:(ct + 1) * P],
                                    vw_sb[e][:, dh * NJ:(dh + 1) * NJ],
                                    start=(e == 0), stop=(e == EC - 1))
                            nc.vector.tensor_add(
                                v_sb[ct][:, dh * NJ:(dh + 1) * NJ], ps[:],
                                vb_h[0:1, dh * NJ:(dh + 1) * NJ]
                                .partition_broadcast(P))

        # ---- Phase A: causal attention per row-block
        with tc.tile_pool(name="e", bufs=2) as e_pool, \
             tc.tile_pool(name="et", bufs=2) as et_pool, \
             tc.tile_pool(name="r", bufs=2) as r_pool, \
             tc.tile_pool(name="os", bufs=2) as os_pool, \
             tc.tile_pool(name="a_s", bufs=2, space="PSUM") as s_pool, \
             tc.tile_pool(name="a_t", bufs=2, space="PSUM") as t_pool, \
             tc.tile_pool(name="a_o", bufs=2, space="PSUM") as o_pool:
            NKC = C // NJ
            for i in range(RB):
                ncols = (i + 1) * P
                njj = (ncols + NJ - 1) // NJ

                etile = e_pool.tile([P, C], DT, tag="E")
                acc = r_pool.tile([P, NKC], F32, tag="acc")
                for jj in range(njj):
                    n = min(NJ, ncols - jj * NJ)
                    ps_s = s_pool.tile([P, NJ], F32, tag="ps_s", name="ps_s")
                    for d in range(DC):
                        nc.tensor.matmul(
                            ps_s[:, :n],
                            qt_sb[d][:, i * P:(i + 1) * P],
                            kt_sb[d][:, jj * NJ:jj * NJ + n],
                            start=(d == 0), stop=(d == DC - 1))
                    if jj == njj - 1:
                        dcol = i * P - jj * NJ
                        nc.vector.tensor_add(ps_s[:, dcol:dcol + P],
                                             ps_s[:, dcol:dcol + P],
                                             cmask[:])
                    nc.scalar.activation(
                        etile[:, jj * NJ:jj * NJ + n], ps_s[:, :n], AF.Exp,
                        scale=scale, accum_out=acc[:, jj:jj + 1])

                rs = r_pool.tile([P, 1], F32, tag="rs")
                nc.vector.reduce_sum(rs[:], acc[:, :njj],
                                     axis=mybir.AxisListType.X)
                rinv = r_pool.tile([P, 1], F32, tag="rinv")
                nc.vector.reciprocal(rinv[:], rs[:])

                ettile = et_pool.tile([P, C], DT, tag="ET")
                for jj in range(njj):
                    n = min(NJ, ncols - jj * NJ)
                    ps_t = t_pool.tile([P, NJ], DT, tag="ps_t")
                    for j in range(n // P):
                        nc.tensor.transpose(
                            ps_t[:, j * P:(j + 1) * P],
                            etile[:, jj * NJ + j * P: jj * NJ + (j + 1) * P],
                            ident_h[:])
                    nc.vector.tensor_copy(ettile[:, jj * NJ:jj * NJ + n],
                                          ps_t[:, :n])

                ps_o = o_pool.tile([P, D], F32, tag="ps_o")
                for dh in range(ND):
                    for j in range(i + 1):
                        nc.tensor.matmul(
                            ps_o[:, dh * NJ:(dh + 1) * NJ],
                            ettile[:, j * P:(j + 1) * P],
                            v_sb[j][:, dh * NJ:(dh + 1) * NJ],
                            start=(j == 0), stop=(j == i))
                outst = os_pool.tile([P, D], F32, tag="outst")
                nc.vector.tensor_scalar_mul(outst[:], ps_o[:], rinv[:])
                nc.sync.dma_start(out_d[i * P:(i + 1) * P, :], outst[:])

    nc.compile()
    return nc


_CACHE = {}


def _built(C=2048, E=1024, D=1024, n_cores=8, loop=1, dt="fp16"):
    key = (C, E, D, n_cores, loop, dt)
    if key not in _CACHE:
        _CACHE[key] = build(C, E, D, n_cores, loop, dt)
    return _CACHE[key]


def _executable(C=2048, E=1024, D=1024, n_cores=8, loop=1, dt="fp16"):
    """Cached jitted SPMD executable for the built Bass module.

    Replicates concourse.bass2jax.run_bass_via_pjrt's multi-core path but
    caches the jit so repeat calls don't retrace, and exposes the pieces
    needed for device-resident benchmarking.
    """
    key = ("exec", C, E, D, n_cores, loop, dt)
    if key in _CACHE:
        return _CACHE[key]
    import jax
    from jax.sharding import Mesh, PartitionSpec
    from jax.experimental.shard_map import shard_map
    from concourse import bass2jax, mybir as _mybir

    nc = _built(C, E, D, n_cores, loop, dt)
    bass2jax.install_neuronx_cc_hook()

    partition_name = (nc.partition_id_tensor.name
                      if nc.partition_id_tensor else None)
    in_names, out_names, out_avals, zero_outs = [], [], [], []
    for alloc in nc.m.functions[0].allocations:
        if not isinstance(alloc, _mybir.MemoryLocationSet):
            continue
        name = alloc.memorylocations[0].name
        if alloc.kind == "ExternalInput":
            if name != partition_name:
                in_names.append(name)
        elif alloc.kind == "ExternalOutput":
            out_names.append(name)
            shape = tuple(alloc.tensor_shape)
            dtype = _mybir.dt.np(alloc.dtype)
            out_avals.append(jax.core.ShapedArray(shape, dtype))
            zero_outs.append(np.zeros(shape, dtype))
    n_params = len(in_names)
    all_names = in_names + out_names
    if partition_name is not None:
        all_names = all_names + [partition_name]

    def _body(*args):
        operands = list(args)
        if partition_name is not None:
            operands.append(bass2jax.partition_id_tensor())
        outs = bass2jax._bass_exec_p.bind(
            *operands,
            out_avals=tuple(out_avals),
            in_names=tuple(all_names),
            out_names=tuple(out_names),
            lowering_input_output_aliases=(),
            sim_require_finite=True,
            sim_require_nnan=True,
            nc=nc,
        )
        return tuple(outs)

    devices = jax.devices()[:n_cores]
    mesh = Mesh(np.asarray(devices), ("core",))
    n_outs = len(out_names)
    sharded = jax.jit(
        shard_map(_body, mesh=mesh,
                  in_specs=(PartitionSpec("core"),) * (n_params + n_outs),
                  out_specs=(PartitionSpec("core"),) * n_outs,
                  check_rep=False),
        donate_argnums=tuple(range(n_params, n_params + n_outs)),
        keep_unused=True,
    )
    res = dict(fn=sharded, in_names=in_names, out_names=out_names,
               out_avals=out_avals, zero_outs=zero_outs, mesh=mesh,
               n_cores=n_cores)
    _CACHE[key] = res
    return res


def run(inputs, C=2048, E=1024, D=1024, n_cores=8, dt="fp16"):
    ex = _executable(C, E, D, n_cores, 1, dt)
    B = inputs["x"].shape[0]
    assert B == n_cores
    f = lambda a: np.ascontiguousarray(np.asarray(a, dtype=np.float32))
    shared = {k: f(inputs[k]) for k in ("Qw", "Qb", "Kw", "Kb", "Vw", "Vb")}
    x = f(inputs["x"])
    per_core = [dict(x=x[b], **shared) for b in range(B)]
    concat_in = [
        np.concatenate([per_core[c][n] for c in range(n_cores)], axis=0)
        for n in ex["in_names"]
    ]
    concat_zeros = [
        np.zeros((n_cores * z.shape[0], *z.shape[1:]), z.dtype)
        for z in ex["zero_outs"]
    ]
    out_arrs = ex["fn"](*concat_in, *concat_zeros)
    i = ex["out_names"].index("out")
    out = np.asarray(out_arrs[i]).reshape(n_cores, *ex["out_avals"][i].shape)
    return out


def kernel(**inputs) -> np.ndarray:
    return run(inputs)
